# revision 30
# baseline (speedup 1.0000x reference)
"""Trainium2 Bass kernel for nn_AttnLayer (dense_transformer, sum-normalized attention).

Reference computation (per batch b, all fp32):
    d      = X @ W1.T + T @ W2.T + (b1+b2)      X=in_seq, T=prev_target_seq
    S      = d @ E.T                            E=enc_seq
    ssum_l = sum_m S[l,m]                       (sum-normalization, NOT softmax)
    out    = (S @ E / ssum[:,None]) @ W3.T + b3

Algorithm (PE computes out = lhsT.T @ rhs, contraction over the partition dim).
The attention is linear, so S is never materialized:
    G  = E.T @ E      [e,e']  bf16 inputs, fp32 psum     (Gram, halves score FLOPs)
    H  = G @ W3T      [e,o]   bf16 x bf16
    dT = W1T.T @ X^T + W2T.T @ T^T + bd   [e,l]  f32r (fp32 bits, fast PE mode)
    O  = dT.T @ H     [l,o]   f32r, then * 1/ssum + b3

ssum numerics: catastrophic cancellation (min |ssum| ~ 0.05 vs typical ~700)
forces the denominator path to exact fp32:
    ssum = X @ v1 + T @ v2 + c0,  v1 = W1.T @ esum,  esum = sum_m E[m,:]
v1/v2/c0 are host-precomputed in fp64 (tiny O(LBD+D^2) input/weight reductions);
on device ssum is 8 fp32 PE matmuls per l-block with lhsT = xT block and
rhs = v1 column, accumulating in a PSUM column -- exact fp32 and the result
lands as a [128,1] per-partition column, directly usable as the output scale.
xT/tT are loaded once as fp32 and bitcast to f32r for the big matmuls (f32r is
fp32 bits; at moving-dim 512 it runs 1 cycle/row, full PE rate), so no tensor
is ever loaded twice. The numerator's bf16/f32r errors stay relative to the
numerator and cancel against the same 1/ssum.

Sharding: data-parallel over batch B=16 across 8 cores (2 batches per core).
All input loads on the Act HWDGE queue in dependency order (Gram inputs first,
then the d-chain t-pass, then x-pass); output stores on the idle Pool SWDGE.
PE is the bottleneck and is kept continuously busy (full 2.4 GHz p-state).
"""

import os

os.environ.setdefault("MYCRO_LOCAL_CACHE", "1")

import numpy as np
import ml_dtypes

import concourse.bass as bass
from concourse import bacc
import concourse.mybir as mybir
import concourse.tile as tile
from concourse.bass_utils import run_bass_kernel_spmd

# Problem shape (hardcoded per contract)
L = 1024      # L_in == L_enc
B = 16
D = 512       # D_in == D_enc == D_emb
N_CORES = 8
BPC = B // N_CORES   # batches per core
P = 128
NE = D // P          # 4 chunks of the 512-wide contraction axes
NM = L // P          # 8 chunks of the L_enc axis
NL = L // P          # 8 chunks of the L_in axis
NLH = 2              # l processed in halves of 512 (moving-operand max for 4-byte)
LH = L // NLH

F32 = mybir.dt.float32
F32R = mybir.dt.float32r
BF16 = mybir.dt.bfloat16
BF16_NP = np.dtype(ml_dtypes.bfloat16)

# vecs packing (columns of the [P, 22] fp32 host-packed vector block):
#   v1 for b: cols [b*NE, b*NE+NE)          (X matvec weights, i on partitions)
#   v2 for b: cols [8 + b*NE, ...)
#   bd:       cols [16, 20)                 (b1+b2, e on partitions)
#   c0 for b: col 20+b                      (esum . bd, replicated)
V1C = 0
V2C = 2 * NE
BDC = 4 * NE
C0C = 4 * NE + NE


def build_nc():
    nc = bacc.Bacc(None, target_bir_lowering=False, debug=False)

    xT_d = nc.declare_dram_parameter("xT", [BPC, D, L], F32, isOutput=False)
    tT_d = nc.declare_dram_parameter("tT", [BPC, D, L], F32, isOutput=False)
    enb_d = nc.declare_dram_parameter("enb", [BPC, L, D], BF16, isOutput=False)
    w1t_d = nc.declare_dram_parameter("w1t", [D, D], F32R, isOutput=False)   # [i, e]
    w2t_d = nc.declare_dram_parameter("w2t", [D, D], F32R, isOutput=False)   # [j, e]
    w3tb_d = nc.declare_dram_parameter("w3tb", [D, D], BF16, isOutput=False)  # [e, o]
    vecs_d = nc.declare_dram_parameter("vecs", [P, 22], F32, isOutput=False)
    b3bc_d = nc.declare_dram_parameter("b3bc", [P, D], F32, isOutput=False)
    out_d = nc.declare_dram_parameter("out", [BPC, L, D], BF16, isOutput=True)

    AF = mybir.ActivationFunctionType
    ALU = mybir.AluOpType

    with tile.TileContext(nc) as tc:
        with (
            tc.tile_pool(name="wpool", bufs=1) as wpool,
            tc.tile_pool(name="big", bufs=1) as big,
            tc.tile_pool(name="opool", bufs=6) as opool,
            tc.tile_pool(name="ps", bufs=1, space="PSUM") as ps,
        ):
            w1t = wpool.tile([P, NE, D], F32R, name="w1t")
            w2t = wpool.tile([P, NE, D], F32R, name="w2t")
            w3tb = wpool.tile([P, NE, D], BF16, name="w3tb")
            vecs = wpool.tile([P, 22], F32, name="vecs")
            b3bc = wpool.tile([P, D], F32, name="b3bc")

            # PE p-state warm-up: two const-AP matmuls at t~100ns pin
            # pe_busy_start early, so every real matmul dispatches with
            # ramp > 3us (full 2.4 GHz). Act warm-up preloads the ~1.3us
            # activation table before the G copies need it.
            c_l = nc.const_aps.tensor(1.0, [P, P], BF16)
            c_r = nc.const_aps.tensor(1.0, [P, D], BF16)
            for tg in ("psA", "psB"):
                warm_ps = ps.tile([P, D], F32, name="warm_ps", tag=tg, bufs=4)
                nc.tensor.matmul(warm_ps, c_l, c_r, start=True, stop=True)
            actwarm = wpool.tile([P, 1], F32, name="actwarm")
            nc.scalar.activation(
                actwarm, nc.const_aps.tensor(1.0, [P, 1], F32), AF.Copy)

            # ---- all big loads up front on the SP HWDGE queue in
            # consumption order (enb1 early: batch 1's Gram is interleaved
            # into batch 0's dT phase); stores follow on the same queue.
            # Weights/vecs on the Pool SWDGE queue, need-by ordered. ----
            enbs, xTs, tTs = [], [], []
            for b in range(BPC):
                enbs.append(big.tile([P, NM, D], BF16, name="enb", tag="enb", bufs=2))
                xTs.append(big.tile([P, NE, L], F32, name="xT", tag="xT", bufs=2))
                tTs.append(big.tile([P, NE, L], F32, name="tT", tag="tT", bufs=2))

            def load_enb(b, chunks=((0, 1), (1, 2), (2, 4), (4, 6), (6, 8))):
                for lo, hi in chunks:
                    nc.sync.dma_start(
                        out=enbs[b][:, lo:hi, :],
                        in_=enb_d[b, lo * P : hi * P, :]
                        .rearrange("(c p) e -> p c e", p=P))

            def load_chunks(dst, src_d, b):
                for k in range(NE):
                    nc.sync.dma_start(
                        out=dst[:, k, :], in_=src_d[b, k * P : (k + 1) * P, :])

            load_enb(0)
            load_chunks(tTs[0], tT_d, 0)
            load_enb(1)
            load_chunks(xTs[0], xT_d, 0)
            load_chunks(tTs[1], tT_d, 1)
            load_chunks(xTs[1], xT_d, 1)
            nc.gpsimd.dma_start(out=vecs, in_=vecs_d[:, :])
            nc.gpsimd.dma_start(out=b3bc, in_=b3bc_d[:, :])
            nc.gpsimd.dma_start(
                out=w3tb, in_=w3tb_d.rearrange("(c p) e -> p c e", p=P))
            for k in range(NE):
                nc.gpsimd.dma_start(
                    out=w2t[:, k, :], in_=w2t_d[k * P : (k + 1) * P, :])
            for k in range(NE):
                nc.gpsimd.dma_start(
                    out=w1t[:, k, :], in_=w1t_d[k * P : (k + 1) * P, :])

            # ---- per-batch state ----
            G_sbs = [big.tile([P, NE, D], BF16, name="G_sb", tag="G", bufs=2)
                     for _ in range(BPC)]
            H_sbs = [big.tile([P, NE, D], F32R, name="H_sb", tag="H", bufs=2)
                     for _ in range(BPC)]
            dTs = [big.tile([P, NE, L], F32R, name="dT", tag="dT", bufs=2)
                   for _ in range(BPC)]
            rcolss = [big.tile([P, NL], F32, name="rcols", tag="rcols", bufs=2)
                      for _ in range(BPC)]
            tTrs = {}
            xTrs = {}

            def tTr_copies(b):
                # f32r rounding of tT chunks on DVE (2 elem/cycle for 4-byte
                # copies); all 4 chunks stay live through both lh passes
                tTrs[b] = [big.tile([P, L], F32R, name="tTr", tag="tTr", bufs=4)
                           for _ in range(NE)]
                for k in range(NE):
                    nc.vector.tensor_copy(tTrs[b][k], tTs[b][:, k, :])

            def xTr_copies(b):
                # f32r rounding of xT chunks on Act
                xTrs[b] = [big.tile([P, L], F32R, name="xTr", tag="xTr", bufs=4)
                           for _ in range(NE)]
                for k in range(NE):
                    nc.scalar.activation(xTrs[b][k], xTs[b][:, k, :], AF.Copy)

            def gram(b, tg):
                # G = E.T @ E, mc-outer so each arriving enb chunk feeds all
                # 4 psum tiles; staggered finish overlaps the Act copies
                enb = enbs[b]
                g_ps = [ps.tile([P, D], F32, name="g_ps", tag=tg, bufs=4)
                        for _ in range(NE)]
                for mc in range(NM - 1):
                    for gc in range(NE):
                        nc.tensor.matmul(
                            g_ps[gc], enb[:, mc, gc * P : (gc + 1) * P],
                            enb[:, mc, :], start=(mc == 0), stop=False)
                for gc in range(NE):
                    nc.tensor.matmul(
                        g_ps[gc], enb[:, NM - 1, gc * P : (gc + 1) * P],
                        enb[:, NM - 1, :], start=False, stop=True)
                    nc.scalar.activation(G_sbs[b][:, gc, :], g_ps[gc], AF.Copy)

            def hphase_hc(b, hc):
                h_ps = ps.tile([P, D], F32, name="h_ps", tag="psB", bufs=4)
                for kc in range(NE):
                    nc.tensor.matmul(
                        h_ps, G_sbs[b][:, kc, hc * P : (hc + 1) * P],
                        w3tb[:, kc, :], start=(kc == 0), stop=(kc == NE - 1))
                nc.scalar.activation(H_sbs[b][:, hc, :], h_ps, AF.Copy)

            def hphase(b):
                # H = G @ W3T
                for hc in range(NE):
                    hphase_hc(b, hc)

            d_pss = {}

            def dt_t(b, lh):
                # t-pass of dT[e, lh-half] = W2T.T @ T^T (+ x-pass later);
                # only 4 PSUM banks per half, so slots free mid-phase
                d_ps = [ps.tile([P, LH], F32, name="d_ps", tag="psA", bufs=4)
                        for _ in range(NE)]
                d_pss[(b, lh)] = d_ps
                for k in range(NE):
                    for ec in range(NE):
                        nc.tensor.matmul(
                            d_ps[ec], w2t[:, k, ec * P : (ec + 1) * P],
                            tTrs[b][k][:, lh * LH : (lh + 1) * LH],
                            start=(k == 0), stop=False)

            def dt_x(b, lh, interleave=None):
                # x-pass + per-tile +bd copy-out (staggered DVE drain);
                # optionally interleaves another phase's chunks (on the other
                # PSUM ring) into the DMA-paced gaps
                d_ps = d_pss[(b, lh)]
                for k in range(NE):
                    if interleave is not None:
                        interleave(k)
                    for ec in range(NE):
                        nc.tensor.matmul(
                            d_ps[ec], w1t[:, k, ec * P : (ec + 1) * P],
                            xTrs[b][k][:, lh * LH : (lh + 1) * LH],
                            start=False, stop=(k == NE - 1))
                        if k == NE - 1:
                            nc.vector.tensor_scalar_add(
                                dTs[b][:, ec, lh * LH : (lh + 1) * LH],
                                d_ps[ec], vecs[:, BDC + ec : BDC + ec + 1])

            def ssum(b):
                # ssum = X@v1 + T@v2 + c0, exact fp32 PE matvecs into PSUM
                # columns; rcols = 1/(ssum+c0) per l-block (DVE). Two s
                # tiles so each slot's readers finish before the ring wraps.
                s_ps = [ps.tile([P, D], F32, name="s_ps", tag="psB", bufs=4)
                        for _ in range(2)]
                for lc in range(NL):
                    cols = slice(lc, lc + 1)
                    scols = slice(lc % 4, lc % 4 + 1)
                    sp = s_ps[lc // 4]
                    i = 0
                    for src, vc0 in ((tTs[b], V2C + b * NE),
                                     (xTs[b], V1C + b * NE)):
                        for k in range(NE):
                            nc.tensor.matmul(
                                sp[:, scols],
                                src[:, k, lc * P : (lc + 1) * P],
                                vecs[:, vc0 + k : vc0 + k + 1],
                                start=(i == 0), stop=(i == 2 * NE - 1))
                            i += 1
                    nc.vector.tensor_scalar_add(
                        rcolss[b][:, cols], sp[:, scols],
                        vecs[:, C0C + b : C0C + b + 1])
                    nc.vector.reciprocal(rcolss[b][:, cols], rcolss[b][:, cols])

            def ophase(b):
                # O = dT.T @ H, then (O * 1/ssum) + b3 fused on DVE with a
                # bf16 output (2 elem/cycle, half the store bytes)
                for lc in range(NL):
                    o_ps = ps.tile([P, D], F32, name="o_ps", tag="psB", bufs=4)
                    for ec in range(NE):
                        nc.tensor.matmul(
                            o_ps, dTs[b][:, ec, lc * P : (lc + 1) * P],
                            H_sbs[b][:, ec, :], start=(ec == 0),
                            stop=(ec == NE - 1))
                    o_sb = opool.tile([P, D], BF16, name="o_sb")
                    nc.vector.scalar_tensor_tensor(
                        o_sb, o_ps, rcolss[b][:, lc : lc + 1], b3bc,
                        op0=ALU.mult, op1=ALU.add)
                    q = nc.sync if lc % 2 == 0 else nc.scalar
                    q.dma_start(
                        out=out_d[b, lc * P : (lc + 1) * P, :], in_=o_sb)

            # ---- interleaved schedule: batch 1's Gram/H fill batch 0's
            # load-paced dT gaps (PSUM ring verified slot-by-slot) ----
            tTr_copies(0)
            gram(0, "psA")
            hphase(0)
            xTr_copies(0)
            dt_t(0, 0)
            gram(1, "psB")
            dt_x(0, 0, interleave=lambda k: hphase_hc(1, k))
            dt_t(0, 1)
            dt_x(0, 1)
            ssum(0)
            tTr_copies(1)
            ophase(0)
            xTr_copies(1)
            dt_t(1, 0)
            dt_x(1, 0)
            dt_t(1, 1)
            dt_x(1, 1)
            ssum(1)
            ophase(1)

    nc.finalize()
    return nc


def _make_in_maps(in_seq, enc_seq, prev_target_seq, W_in2enc, b_in2enc,
                  W_lab2enc, b_lab2enc, W_enc2in, b_enc2in):
    f32 = np.float32
    f64 = np.float64
    W1 = np.asarray(W_in2enc, f32)
    W2 = np.asarray(W_lab2enc, f32)
    W3 = np.asarray(W_enc2in, f32)
    bd = (np.asarray(b_in2enc, f64) + np.asarray(b_lab2enc, f64)).astype(f32)
    w1t = np.ascontiguousarray(W1.T)   # [i, e]
    w2t = np.ascontiguousarray(W2.T)   # [j, e]
    w3tb = np.ascontiguousarray(W3.T.astype(BF16_NP))  # [e, o]
    b3bc = np.ascontiguousarray(
        np.broadcast_to(np.asarray(b_enc2in, f32), (P, D)))

    in_maps = []
    for c in range(N_CORES):
        bs = slice(c * BPC, (c + 1) * BPC)
        x = np.asarray(in_seq[:, bs, :], f32)
        t = np.asarray(prev_target_seq[:, bs, :], f32)
        e = np.asarray(enc_seq[:, bs, :], f32)
        # host glue: esum/v1/v2/c0 in fp64 (O(LBD + D^2) input/weight
        # reductions -- the ssum = X@v1 + T@v2 + c0 reformulation)
        esum = e.astype(f64).sum(axis=0)                    # [BPC, D]
        v1 = (esum @ W1.astype(f64)).astype(f32)            # [BPC, D]
        v2 = (esum @ W2.astype(f64)).astype(f32)
        c0 = (esum @ (np.asarray(b_in2enc, f64) + np.asarray(b_lab2enc, f64))
              ).astype(f32)                                  # [BPC]
        vecs = np.zeros((P, 22), f32)
        for b in range(BPC):
            vecs[:, V1C + b * NE : V1C + (b + 1) * NE] = v1[b].reshape(NE, P).T
            vecs[:, V2C + b * NE : V2C + (b + 1) * NE] = v2[b].reshape(NE, P).T
            vecs[:, C0C + b] = c0[b]
        vecs[:, BDC : BDC + NE] = bd.reshape(NE, P).T
        in_maps.append({
            "xT": np.ascontiguousarray(x.transpose(1, 2, 0)),
            "tT": np.ascontiguousarray(t.transpose(1, 2, 0)),
            "enb": np.ascontiguousarray(e.transpose(1, 0, 2).astype(BF16_NP)),
            "w1t": w1t, "w2t": w2t, "w3tb": w3tb,
            "vecs": vecs, "b3bc": b3bc,
        })
    return in_maps


_NC_CACHE = {}


def _get_nc():
    if "nc" not in _NC_CACHE:
        _NC_CACHE["nc"] = build_nc()
    return _NC_CACHE["nc"]


def kernel(**inputs):
    in_maps = _make_in_maps(**inputs)
    nc = _get_nc()
    res = run_bass_kernel_spmd(nc, in_maps, core_ids=list(range(N_CORES)))
    out = np.empty((L, B, D), np.float32)
    for c in range(N_CORES):
        per_core = np.asarray(res.results[c]["out"], np.float32)  # (BPC, L, D)
        for j in range(BPC):
            out[:, c * BPC + j, :] = per_core[j]
    return out


def kernel_sim(core_id=0, **inputs):
    """CoreSim validation path: simulate one core, return its (BPC, L, D) output."""
    from concourse.bass_interp import CoreSim

    in_maps = _make_in_maps(**inputs)
    nc = _get_nc()
    sim = CoreSim(nc)
    for name, val in in_maps[core_id].items():
        sim.tensor(name)[:] = val
    sim.simulate(check_with_hw=False)
    return np.array(sim.tensor("out")).astype(np.float32)


# revision 31
# speedup vs baseline: 1.1331x; 1.1331x over previous
"""Trainium2 Bass kernel for nn_AttnLayer (dense_transformer, sum-normalized attention).

Reference computation (per batch b, all fp32):
    d      = X @ W1.T + T @ W2.T + (b1+b2)      X=in_seq, T=prev_target_seq
    S      = d @ E.T                            E=enc_seq
    ssum_l = sum_m S[l,m]                       (sum-normalization, NOT softmax)
    out    = (S @ E / ssum[:,None]) @ W3.T + b3

Algorithm (PE computes out = lhsT.T @ rhs, contraction over the partition dim).
The attention is linear, so S is never materialized:
    G  = E.T @ E      [e,e']  bf16 inputs, fp32 psum     (Gram, halves score FLOPs)
    H  = G @ W3T      [e,o]   bf16 x bf16
    dT = W1T.T @ X^T + W2T.T @ T^T + bd   [e,l]  f32r (fp32 bits, fast PE mode)
    O  = dT.T @ H     [l,o]   f32r, then * 1/ssum + b3

ssum numerics: catastrophic cancellation (min |ssum| ~ 0.05 vs typical ~700)
forces the denominator path to exact fp32:
    ssum = X @ v1 + T @ v2 + c0,  v1 = W1.T @ esum,  esum = sum_m E[m,:]
v1/v2/c0 are host-precomputed in fp64 (tiny O(LBD+D^2) input/weight reductions);
on device ssum is 8 fp32 PE matmuls per l-block with lhsT = xT block and
rhs = v1 column, accumulating in a PSUM column -- exact fp32 and the result
lands as a [128,1] per-partition column, directly usable as the output scale.
xT/tT are loaded once as fp32 and bitcast to f32r for the big matmuls (f32r is
fp32 bits; at moving-dim 512 it runs 1 cycle/row, full PE rate), so no tensor
is ever loaded twice. The numerator's bf16/f32r errors stay relative to the
numerator and cancel against the same 1/ssum.

Sharding: data-parallel over batch B=16 across 8 cores (2 batches per core).
All input loads on the Act HWDGE queue in dependency order (Gram inputs first,
then the d-chain t-pass, then x-pass); output stores on the idle Pool SWDGE.
PE is the bottleneck and is kept continuously busy (full 2.4 GHz p-state).
"""

import os

os.environ.setdefault("MYCRO_LOCAL_CACHE", "1")

import numpy as np
import ml_dtypes

import concourse.bass as bass
from concourse import bacc
import concourse.mybir as mybir
import concourse.tile as tile
from concourse.bass_utils import run_bass_kernel_spmd

# Problem shape (hardcoded per contract)
L = 1024      # L_in == L_enc
B = 16
D = 512       # D_in == D_enc == D_emb
N_CORES = 8
BPC = B // N_CORES   # batches per core
P = 128
NE = D // P          # 4 chunks of the 512-wide contraction axes
NM = L // P          # 8 chunks of the L_enc axis
NL = L // P          # 8 chunks of the L_in axis
NLH = 2              # l processed in halves of 512 (moving-operand max for 4-byte)
LH = L // NLH

F32 = mybir.dt.float32
F32R = mybir.dt.float32r
BF16 = mybir.dt.bfloat16
BF16_NP = np.dtype(ml_dtypes.bfloat16)

# vecs packing (columns of the [P, 22] fp32 host-packed vector block):
#   v1 for b: cols [b*NE, b*NE+NE)          (X matvec weights, i on partitions)
#   v2 for b: cols [8 + b*NE, ...)
#   bd:       cols [16, 20)                 (b1+b2, e on partitions)
#   c0 for b: col 20+b                      (esum . bd, replicated)
V1C = 0
V2C = 2 * NE
BDC = 4 * NE
C0C = 4 * NE + NE


def build_nc():
    nc = bacc.Bacc(None, target_bir_lowering=False, debug=False)

    xT_d = nc.declare_dram_parameter("xT", [BPC, D, L], F32, isOutput=False)
    tT_d = nc.declare_dram_parameter("tT", [BPC, D, L], F32, isOutput=False)
    enb_d = nc.declare_dram_parameter("enb", [BPC, L, D], BF16, isOutput=False)
    w1t_d = nc.declare_dram_parameter("w1t", [D, D], F32R, isOutput=False)   # [i, e]
    w2t_d = nc.declare_dram_parameter("w2t", [D, D], F32R, isOutput=False)   # [j, e]
    w3tb_d = nc.declare_dram_parameter("w3tb", [D, D], BF16, isOutput=False)  # [e, o]
    vecs_d = nc.declare_dram_parameter("vecs", [P, 22], F32, isOutput=False)
    b3bc_d = nc.declare_dram_parameter("b3bc", [P, D], F32, isOutput=False)
    out_d = nc.declare_dram_parameter("out", [BPC, L, D], BF16, isOutput=True)

    AF = mybir.ActivationFunctionType
    ALU = mybir.AluOpType

    with tile.TileContext(nc) as tc:
        with (
            tc.tile_pool(name="wpool", bufs=1) as wpool,
            tc.tile_pool(name="big", bufs=1) as big,
            tc.tile_pool(name="opool", bufs=6) as opool,
            tc.tile_pool(name="ps", bufs=1, space="PSUM") as ps,
        ):
            w1t = wpool.tile([P, NE, D], F32R, name="w1t")
            w2t = wpool.tile([P, NE, D], F32R, name="w2t")
            w3tb = wpool.tile([P, NE, D], BF16, name="w3tb")
            vecs = wpool.tile([P, 22], F32, name="vecs")
            b3bc = wpool.tile([P, D], F32, name="b3bc")

            # PE p-state warm-up: two const-AP matmuls at t~100ns pin
            # pe_busy_start early, so every real matmul dispatches with
            # ramp > 3us (full 2.4 GHz). Act warm-up preloads the ~1.3us
            # activation table before the G copies need it.
            c_l = nc.const_aps.tensor(1.0, [P, P], BF16)
            c_r = nc.const_aps.tensor(1.0, [P, D], BF16)
            for _ in range(2):
                warm_ps = ps.tile([P, D], F32, name="warm_ps", tag="ps8", bufs=8)
                nc.tensor.matmul(warm_ps, c_l, c_r, start=True, stop=True)
            actwarm = wpool.tile([P, 1], F32, name="actwarm")
            nc.scalar.activation(
                actwarm, nc.const_aps.tensor(1.0, [P, 1], F32), AF.Copy)

            # ---- all big loads up front on the SP HWDGE queue in
            # consumption order (enb1 early: batch 1's Gram is interleaved
            # into batch 0's dT phase); stores follow on the same queue.
            # Weights/vecs on the Pool SWDGE queue, need-by ordered. ----
            enbs, xTs, tTs = [], [], []
            for b in range(BPC):
                enbs.append(big.tile([P, NM, D], BF16, name="enb", tag="enb", bufs=2))
                xTs.append(big.tile([P, NE, L], F32, name="xT", tag="xT", bufs=2))
                tTs.append(big.tile([P, NE, L], F32, name="tT", tag="tT", bufs=2))

            def load_enb(b, chunks=((0, 1), (1, 2), (2, 4), (4, 6), (6, 8))):
                for lo, hi in chunks:
                    nc.sync.dma_start(
                        out=enbs[b][:, lo:hi, :],
                        in_=enb_d[b, lo * P : hi * P, :]
                        .rearrange("(c p) e -> p c e", p=P))

            def load_chunks(dst, src_d, b):
                for k in range(NE):
                    nc.sync.dma_start(
                        out=dst[:, k, :], in_=src_d[b, k * P : (k + 1) * P, :])

            load_enb(0)
            load_chunks(tTs[0], tT_d, 0)
            load_enb(1)
            load_chunks(xTs[0], xT_d, 0)
            load_chunks(tTs[1], tT_d, 1)
            load_chunks(xTs[1], xT_d, 1)
            nc.gpsimd.dma_start(out=vecs, in_=vecs_d[:, :])
            nc.gpsimd.dma_start(out=b3bc, in_=b3bc_d[:, :])
            nc.gpsimd.dma_start(
                out=w3tb, in_=w3tb_d.rearrange("(c p) e -> p c e", p=P))
            for k in range(NE):
                nc.gpsimd.dma_start(
                    out=w2t[:, k, :], in_=w2t_d[k * P : (k + 1) * P, :])
            for k in range(NE):
                nc.gpsimd.dma_start(
                    out=w1t[:, k, :], in_=w1t_d[k * P : (k + 1) * P, :])

            # ---- per-batch state ----
            G_sbs = [big.tile([P, NE, D], BF16, name="G_sb", tag="G", bufs=2)
                     for _ in range(BPC)]
            H_sbs = [big.tile([P, NE, D], F32R, name="H_sb", tag="H", bufs=2)
                     for _ in range(BPC)]
            dTs = [big.tile([P, NE, L], F32R, name="dT", tag="dT", bufs=2)
                   for _ in range(BPC)]
            rcolss = [big.tile([P, NL], F32, name="rcols", tag="rcols", bufs=2)
                      for _ in range(BPC)]
            tTrs = {}
            xTrs = {}

            def tTr_copies(b):
                # f32r rounding of tT chunks on DVE (2 elem/cycle for 4-byte
                # copies); all 4 chunks stay live through both lh passes
                tTrs[b] = [big.tile([P, L], F32R, name="tTr", tag="tTr", bufs=4)
                           for _ in range(NE)]
                for k in range(NE):
                    nc.vector.tensor_copy(tTrs[b][k], tTs[b][:, k, :])

            def xTr_copies(b):
                # f32r rounding of xT chunks on Act
                xTrs[b] = [big.tile([P, L], F32R, name="xTr", tag="xTr", bufs=4)
                           for _ in range(NE)]
                for k in range(NE):
                    nc.scalar.activation(xTrs[b][k], xTs[b][:, k, :], AF.Copy)

            def gram(b):
                # G = E.T @ E, mc-outer so each arriving enb chunk feeds all
                # 4 psum tiles; staggered finish overlaps the Act copies
                enb = enbs[b]
                g_ps = [ps.tile([P, D], F32, name="g_ps", tag="ps8", bufs=8)
                        for _ in range(NE)]
                for mc in range(NM - 1):
                    for gc in range(NE):
                        nc.tensor.matmul(
                            g_ps[gc], enb[:, mc, gc * P : (gc + 1) * P],
                            enb[:, mc, :], start=(mc == 0), stop=False)
                for gc in range(NE):
                    nc.tensor.matmul(
                        g_ps[gc], enb[:, NM - 1, gc * P : (gc + 1) * P],
                        enb[:, NM - 1, :], start=False, stop=True)
                    nc.scalar.activation(G_sbs[b][:, gc, :], g_ps[gc], AF.Copy)

            def hphase_hc(b, hc):
                h_ps = ps.tile([P, D], F32, name="h_ps", tag="ps8", bufs=8)
                for kc in range(NE):
                    nc.tensor.matmul(
                        h_ps, G_sbs[b][:, kc, hc * P : (hc + 1) * P],
                        w3tb[:, kc, :], start=(kc == 0), stop=(kc == NE - 1))
                nc.scalar.activation(H_sbs[b][:, hc, :], h_ps, AF.Copy)

            def hphase(b):
                # H = G @ W3T
                for hc in range(NE):
                    hphase_hc(b, hc)

            d_pss = {}

            def dt_t(b, lh):
                # t-pass of dT[e, lh-half] = W2T.T @ T^T (+ x-pass later);
                # only 4 PSUM banks per half, so slots free mid-phase
                d_ps = [ps.tile([P, LH], F32, name="d_ps", tag="ps8", bufs=8)
                        for _ in range(NE)]
                d_pss[(b, lh)] = d_ps
                for k in range(NE):
                    for ec in range(NE):
                        nc.tensor.matmul(
                            d_ps[ec], w2t[:, k, ec * P : (ec + 1) * P],
                            tTrs[b][k][:, lh * LH : (lh + 1) * LH],
                            start=(k == 0), stop=False)

            def dt_x(b, lh, interleave=None):
                # x-pass + per-tile +bd copy-out (staggered DVE drain);
                # optionally interleaves another phase's chunks (on the other
                # PSUM ring) into the DMA-paced gaps
                d_ps = d_pss[(b, lh)]
                for k in range(NE):
                    if interleave is not None:
                        interleave(k)
                    for ec in range(NE):
                        nc.tensor.matmul(
                            d_ps[ec], w1t[:, k, ec * P : (ec + 1) * P],
                            xTrs[b][k][:, lh * LH : (lh + 1) * LH],
                            start=False, stop=(k == NE - 1))
                        if k == NE - 1:
                            nc.vector.tensor_scalar_add(
                                dTs[b][:, ec, lh * LH : (lh + 1) * LH],
                                d_ps[ec], vecs[:, BDC + ec : BDC + ec + 1])

            def ssum(b):
                # ssum = X@v1 + T@v2 + c0, exact fp32 PE matvecs into PSUM
                # columns; rcols = 1/(ssum+c0) per l-block (DVE). Two s
                # tiles so each slot's readers finish before the ring wraps.
                s_ps = [ps.tile([P, D], F32, name="s_ps", tag="ps8", bufs=8)
                        for _ in range(2)]
                for lc in range(NL):
                    cols = slice(lc, lc + 1)
                    scols = slice(lc % 4, lc % 4 + 1)
                    sp = s_ps[lc // 4]
                    i = 0
                    for src, vc0 in ((tTs[b], V2C + b * NE),
                                     (xTs[b], V1C + b * NE)):
                        for k in range(NE):
                            nc.tensor.matmul(
                                sp[:, scols],
                                src[:, k, lc * P : (lc + 1) * P],
                                vecs[:, vc0 + k : vc0 + k + 1],
                                start=(i == 0), stop=(i == 2 * NE - 1))
                            i += 1
                    nc.vector.tensor_scalar_add(
                        rcolss[b][:, cols], sp[:, scols],
                        vecs[:, C0C + b : C0C + b + 1])
                    nc.vector.reciprocal(rcolss[b][:, cols], rcolss[b][:, cols])

            def ophase(b):
                # O = dT.T @ H, then (O * 1/ssum) + b3 fused on DVE with a
                # bf16 output (2 elem/cycle, half the store bytes)
                for lc in range(NL):
                    o_ps = ps.tile([P, D], F32, name="o_ps", tag="ps8", bufs=8)
                    for ec in range(NE):
                        nc.tensor.matmul(
                            o_ps, dTs[b][:, ec, lc * P : (lc + 1) * P],
                            H_sbs[b][:, ec, :], start=(ec == 0),
                            stop=(ec == NE - 1))
                    o_sb = opool.tile([P, D], BF16, name="o_sb")
                    nc.vector.scalar_tensor_tensor(
                        o_sb, o_ps, rcolss[b][:, lc : lc + 1], b3bc,
                        op0=ALU.mult, op1=ALU.add)
                    q = nc.sync if lc % 2 == 0 else nc.scalar
                    q.dma_start(
                        out=out_d[b, lc * P : (lc + 1) * P, :], in_=o_sb)

            # ---- interleaved schedule: batch 1's Gram/H fill batch 0's
            # load-paced dT gaps (PSUM ring verified slot-by-slot) ----
            tTr_copies(0)
            gram(0)
            hphase(0)
            xTr_copies(0)
            dt_t(0, 0)
            gram(1)
            dt_x(0, 0)
            hphase(1)
            dt_t(0, 1)
            dt_x(0, 1)
            ssum(0)
            tTr_copies(1)
            ophase(0)
            xTr_copies(1)
            dt_t(1, 0)
            dt_x(1, 0)
            dt_t(1, 1)
            dt_x(1, 1)
            ssum(1)
            ophase(1)

    nc.finalize()
    return nc


def _make_in_maps(in_seq, enc_seq, prev_target_seq, W_in2enc, b_in2enc,
                  W_lab2enc, b_lab2enc, W_enc2in, b_enc2in):
    f32 = np.float32
    f64 = np.float64
    W1 = np.asarray(W_in2enc, f32)
    W2 = np.asarray(W_lab2enc, f32)
    W3 = np.asarray(W_enc2in, f32)
    bd = (np.asarray(b_in2enc, f64) + np.asarray(b_lab2enc, f64)).astype(f32)
    w1t = np.ascontiguousarray(W1.T)   # [i, e]
    w2t = np.ascontiguousarray(W2.T)   # [j, e]
    w3tb = np.ascontiguousarray(W3.T.astype(BF16_NP))  # [e, o]
    b3bc = np.ascontiguousarray(
        np.broadcast_to(np.asarray(b_enc2in, f32), (P, D)))

    in_maps = []
    for c in range(N_CORES):
        bs = slice(c * BPC, (c + 1) * BPC)
        x = np.asarray(in_seq[:, bs, :], f32)
        t = np.asarray(prev_target_seq[:, bs, :], f32)
        e = np.asarray(enc_seq[:, bs, :], f32)
        # host glue: esum/v1/v2/c0 in fp64 (O(LBD + D^2) input/weight
        # reductions -- the ssum = X@v1 + T@v2 + c0 reformulation)
        esum = e.astype(f64).sum(axis=0)                    # [BPC, D]
        v1 = (esum @ W1.astype(f64)).astype(f32)            # [BPC, D]
        v2 = (esum @ W2.astype(f64)).astype(f32)
        c0 = (esum @ (np.asarray(b_in2enc, f64) + np.asarray(b_lab2enc, f64))
              ).astype(f32)                                  # [BPC]
        vecs = np.zeros((P, 22), f32)
        for b in range(BPC):
            vecs[:, V1C + b * NE : V1C + (b + 1) * NE] = v1[b].reshape(NE, P).T
            vecs[:, V2C + b * NE : V2C + (b + 1) * NE] = v2[b].reshape(NE, P).T
            vecs[:, C0C + b] = c0[b]
        vecs[:, BDC : BDC + NE] = bd.reshape(NE, P).T
        in_maps.append({
            "xT": np.ascontiguousarray(x.transpose(1, 2, 0)),
            "tT": np.ascontiguousarray(t.transpose(1, 2, 0)),
            "enb": np.ascontiguousarray(e.transpose(1, 0, 2).astype(BF16_NP)),
            "w1t": w1t, "w2t": w2t, "w3tb": w3tb,
            "vecs": vecs, "b3bc": b3bc,
        })
    return in_maps


_NC_CACHE = {}


def _get_nc():
    if "nc" not in _NC_CACHE:
        _NC_CACHE["nc"] = build_nc()
    return _NC_CACHE["nc"]


def kernel(**inputs):
    in_maps = _make_in_maps(**inputs)
    nc = _get_nc()
    res = run_bass_kernel_spmd(nc, in_maps, core_ids=list(range(N_CORES)))
    out = np.empty((L, B, D), np.float32)
    for c in range(N_CORES):
        per_core = np.asarray(res.results[c]["out"], np.float32)  # (BPC, L, D)
        for j in range(BPC):
            out[:, c * BPC + j, :] = per_core[j]
    return out


def kernel_sim(core_id=0, **inputs):
    """CoreSim validation path: simulate one core, return its (BPC, L, D) output."""
    from concourse.bass_interp import CoreSim

    in_maps = _make_in_maps(**inputs)
    nc = _get_nc()
    sim = CoreSim(nc)
    for name, val in in_maps[core_id].items():
        sim.tensor(name)[:] = val
    sim.simulate(check_with_hw=False)
    return np.array(sim.tensor("out")).astype(np.float32)


# revision 32
# speedup vs baseline: 1.1456x; 1.0110x over previous
"""Trainium2 Bass kernel for nn_AttnLayer (dense_transformer, sum-normalized attention).

Reference computation (per batch b, all fp32):
    d      = X @ W1.T + T @ W2.T + (b1+b2)      X=in_seq, T=prev_target_seq
    S      = d @ E.T                            E=enc_seq
    ssum_l = sum_m S[l,m]                       (sum-normalization, NOT softmax)
    out    = (S @ E / ssum[:,None]) @ W3.T + b3

Algorithm (PE computes out = lhsT.T @ rhs, contraction over the partition dim).
The attention is linear, so S is never materialized:
    G  = E.T @ E      [e,e']  bf16 inputs, fp32 psum     (Gram, halves score FLOPs)
    H  = G @ W3T      [e,o]   bf16 x bf16
    dT = W1T.T @ X^T + W2T.T @ T^T + bd   [e,l]  f32r (fp32 bits, fast PE mode)
    O  = dT.T @ H     [l,o]   f32r, then * 1/ssum + b3

ssum numerics: catastrophic cancellation (min |ssum| ~ 0.05 vs typical ~700)
forces the denominator path to exact fp32:
    ssum = X @ v1 + T @ v2 + c0,  v1 = W1.T @ esum,  esum = sum_m E[m,:]
v1/v2/c0 are host-precomputed in fp64 (tiny O(LBD+D^2) input/weight reductions);
on device ssum is 8 fp32 PE matmuls per l-block with lhsT = xT block and
rhs = v1 column, accumulating in a PSUM column -- exact fp32 and the result
lands as a [128,1] per-partition column, directly usable as the output scale.
xT/tT are loaded once as fp32 and bitcast to f32r for the big matmuls (f32r is
fp32 bits; at moving-dim 512 it runs 1 cycle/row, full PE rate), so no tensor
is ever loaded twice. The numerator's bf16/f32r errors stay relative to the
numerator and cancel against the same 1/ssum.

Sharding: data-parallel over batch B=16 across 8 cores (2 batches per core).
All input loads on the Act HWDGE queue in dependency order (Gram inputs first,
then the d-chain t-pass, then x-pass); output stores on the idle Pool SWDGE.
PE is the bottleneck and is kept continuously busy (full 2.4 GHz p-state).
"""

import os

os.environ.setdefault("MYCRO_LOCAL_CACHE", "1")

import numpy as np
import ml_dtypes

import concourse.bass as bass
from concourse import bacc
import concourse.mybir as mybir
import concourse.tile as tile
from concourse.bass_utils import run_bass_kernel_spmd

# Problem shape (hardcoded per contract)
L = 1024      # L_in == L_enc
B = 16
D = 512       # D_in == D_enc == D_emb
N_CORES = 8
BPC = B // N_CORES   # batches per core
P = 128
NE = D // P          # 4 chunks of the 512-wide contraction axes
NM = L // P          # 8 chunks of the L_enc axis
NL = L // P          # 8 chunks of the L_in axis
NLH = 2              # l processed in halves of 512 (moving-operand max for 4-byte)
LH = L // NLH

F32 = mybir.dt.float32
F32R = mybir.dt.float32r
BF16 = mybir.dt.bfloat16
BF16_NP = np.dtype(ml_dtypes.bfloat16)

# vecs packing (columns of the [P, 22] fp32 host-packed vector block):
#   v1 for b: cols [b*NE, b*NE+NE)          (X matvec weights, i on partitions)
#   v2 for b: cols [8 + b*NE, ...)
#   bd:       cols [16, 20)                 (b1+b2, e on partitions)
#   c0 for b: col 20+b                      (esum . bd, replicated)
V1C = 0
V2C = 2 * NE
BDC = 4 * NE
C0C = 4 * NE + NE


def build_nc():
    nc = bacc.Bacc(None, target_bir_lowering=False, debug=False)

    xT_d = nc.declare_dram_parameter("xT", [BPC, D, L], F32, isOutput=False)
    tT_d = nc.declare_dram_parameter("tT", [BPC, D, L], F32, isOutput=False)
    enb_d = nc.declare_dram_parameter("enb", [BPC, L, D], BF16, isOutput=False)
    w1t_d = nc.declare_dram_parameter("w1t", [D, D], F32R, isOutput=False)   # [i, e]
    w2t_d = nc.declare_dram_parameter("w2t", [D, D], F32R, isOutput=False)   # [j, e]
    w3tb_d = nc.declare_dram_parameter("w3tb", [D, D], BF16, isOutput=False)  # [e, o]
    vecs_d = nc.declare_dram_parameter("vecs", [P, 22], F32, isOutput=False)
    b3bc_d = nc.declare_dram_parameter("b3bc", [P, D], F32, isOutput=False)
    out_d = nc.declare_dram_parameter("out", [BPC, L, D], BF16, isOutput=True)

    AF = mybir.ActivationFunctionType
    ALU = mybir.AluOpType

    with tile.TileContext(nc) as tc:
        with (
            tc.tile_pool(name="wpool", bufs=1) as wpool,
            tc.tile_pool(name="big", bufs=1) as big,
            tc.tile_pool(name="opool", bufs=6) as opool,
            tc.tile_pool(name="ps", bufs=1, space="PSUM") as ps,
        ):
            w1t = wpool.tile([P, NE, D], F32R, name="w1t")
            w2t = wpool.tile([P, NE, D], F32R, name="w2t")
            w3tb = wpool.tile([P, NE, D], BF16, name="w3tb")
            vecs = wpool.tile([P, 22], F32, name="vecs")
            b3bc = wpool.tile([P, D], F32, name="b3bc")

            # PE p-state warm-up: two const-AP matmuls at t~100ns pin
            # pe_busy_start early, so every real matmul dispatches with
            # ramp > 3us (full 2.4 GHz). Act warm-up preloads the ~1.3us
            # activation table before the G copies need it.
            c_l = nc.const_aps.tensor(1.0, [P, P], BF16)
            c_r = nc.const_aps.tensor(1.0, [P, D], BF16)
            for _ in range(2):
                warm_ps = ps.tile([P, D], F32, name="warm_ps", tag="ps8", bufs=8)
                nc.tensor.matmul(warm_ps, c_l, c_r, start=True, stop=True)
            actwarm = wpool.tile([P, 1], F32, name="actwarm")
            nc.scalar.activation(
                actwarm, nc.const_aps.tensor(1.0, [P, 1], F32), AF.Copy)

            # ---- all big loads up front on the SP HWDGE queue in
            # consumption order (enb1 early: batch 1's Gram is interleaved
            # into batch 0's dT phase); stores follow on the same queue.
            # Weights/vecs on the Pool SWDGE queue, need-by ordered. ----
            enbs, xTs, tTs = [], [], []
            for b in range(BPC):
                enbs.append(big.tile([P, NM, D], BF16, name="enb", tag="enb", bufs=2))
                xTs.append(big.tile([P, NE, L], F32, name="xT", tag="xT", bufs=2))
                tTs.append(big.tile([P, NE, L], F32, name="tT", tag="tT", bufs=2))

            def load_enb(b, chunks=((0, 2), (2, 4), (4, 6), (6, 8))):
                for lo, hi in chunks:
                    nc.sync.dma_start(
                        out=enbs[b][:, lo:hi, :],
                        in_=enb_d[b, lo * P : hi * P, :]
                        .rearrange("(c p) e -> p c e", p=P))

            def load_chunks(dst, src_d, b):
                for k in range(NE):
                    nc.sync.dma_start(
                        out=dst[:, k, :], in_=src_d[b, k * P : (k + 1) * P, :])

            load_enb(0)
            load_chunks(tTs[0], tT_d, 0)
            load_enb(1)
            load_chunks(xTs[0], xT_d, 0)
            load_chunks(tTs[1], tT_d, 1)
            load_chunks(xTs[1], xT_d, 1)
            nc.gpsimd.dma_start(out=vecs, in_=vecs_d[:, :])
            nc.gpsimd.dma_start(out=b3bc, in_=b3bc_d[:, :])
            nc.gpsimd.dma_start(
                out=w3tb, in_=w3tb_d.rearrange("(c p) e -> p c e", p=P))
            for k in range(NE):
                nc.gpsimd.dma_start(
                    out=w2t[:, k, :], in_=w2t_d[k * P : (k + 1) * P, :])
            for k in range(NE):
                nc.gpsimd.dma_start(
                    out=w1t[:, k, :], in_=w1t_d[k * P : (k + 1) * P, :])

            # ---- per-batch state ----
            G_sbs = [big.tile([P, NE, D], BF16, name="G_sb", tag="G", bufs=2)
                     for _ in range(BPC)]
            H_sbs = [big.tile([P, NE, D], F32R, name="H_sb", tag="H", bufs=2)
                     for _ in range(BPC)]
            dTs = [big.tile([P, NE, L], F32R, name="dT", tag="dT", bufs=2)
                   for _ in range(BPC)]
            rcolss = [big.tile([P, NL], F32, name="rcols", tag="rcols", bufs=2)
                      for _ in range(BPC)]
            tTrs = {}
            xTrs = {}

            def tTr_copies(b):
                # f32r rounding of tT chunks on DVE (2 elem/cycle for 4-byte
                # copies); all 4 chunks stay live through both lh passes
                tTrs[b] = [big.tile([P, L], F32R, name="tTr", tag="tTr", bufs=4)
                           for _ in range(NE)]
                for k in range(NE):
                    nc.vector.tensor_copy(tTrs[b][k], tTs[b][:, k, :])

            def xTr_copies(b):
                # f32r rounding of xT chunks on Act
                xTrs[b] = [big.tile([P, L], F32R, name="xTr", tag="xTr", bufs=4)
                           for _ in range(NE)]
                for k in range(NE):
                    nc.scalar.activation(xTrs[b][k], xTs[b][:, k, :], AF.Copy)

            def gram(b):
                # G = E.T @ E, mc-outer so each arriving enb chunk feeds all
                # 4 psum tiles; staggered finish overlaps the Act copies
                enb = enbs[b]
                g_ps = [ps.tile([P, D], F32, name="g_ps", tag="ps8", bufs=8)
                        for _ in range(NE)]
                for mc in range(NM - 1):
                    for gc in range(NE):
                        nc.tensor.matmul(
                            g_ps[gc], enb[:, mc, gc * P : (gc + 1) * P],
                            enb[:, mc, :], start=(mc == 0), stop=False)
                for gc in range(NE):
                    nc.tensor.matmul(
                        g_ps[gc], enb[:, NM - 1, gc * P : (gc + 1) * P],
                        enb[:, NM - 1, :], start=False, stop=True)
                    nc.scalar.activation(G_sbs[b][:, gc, :], g_ps[gc], AF.Copy)

            def hphase_hc(b, hc):
                h_ps = ps.tile([P, D], F32, name="h_ps", tag="ps8", bufs=8)
                for kc in range(NE):
                    nc.tensor.matmul(
                        h_ps, G_sbs[b][:, kc, hc * P : (hc + 1) * P],
                        w3tb[:, kc, :], start=(kc == 0), stop=(kc == NE - 1))
                nc.scalar.activation(H_sbs[b][:, hc, :], h_ps, AF.Copy)

            def hphase(b):
                # H = G @ W3T
                for hc in range(NE):
                    hphase_hc(b, hc)

            d_pss = {}

            def dt_t(b, lh):
                # t-pass of dT[e, lh-half] = W2T.T @ T^T (+ x-pass later);
                # only 4 PSUM banks per half, so slots free mid-phase
                d_ps = [ps.tile([P, LH], F32, name="d_ps", tag="ps8", bufs=8)
                        for _ in range(NE)]
                d_pss[(b, lh)] = d_ps
                for k in range(NE):
                    for ec in range(NE):
                        nc.tensor.matmul(
                            d_ps[ec], w2t[:, k, ec * P : (ec + 1) * P],
                            tTrs[b][k][:, lh * LH : (lh + 1) * LH],
                            start=(k == 0), stop=False)

            def dt_x(b, lh, interleave=None):
                # x-pass + per-tile +bd copy-out (staggered DVE drain);
                # optionally interleaves another phase's chunks (on the other
                # PSUM ring) into the DMA-paced gaps
                d_ps = d_pss[(b, lh)]
                for k in range(NE):
                    if interleave is not None:
                        interleave(k)
                    for ec in range(NE):
                        nc.tensor.matmul(
                            d_ps[ec], w1t[:, k, ec * P : (ec + 1) * P],
                            xTrs[b][k][:, lh * LH : (lh + 1) * LH],
                            start=False, stop=(k == NE - 1))
                        if k == NE - 1:
                            nc.vector.tensor_scalar_add(
                                dTs[b][:, ec, lh * LH : (lh + 1) * LH],
                                d_ps[ec], vecs[:, BDC + ec : BDC + ec + 1])

            def ssum(b):
                # ssum = X@v1 + T@v2 + c0, exact fp32 PE matvecs into PSUM
                # columns; rcols = 1/(ssum+c0) per l-block (DVE). Two s
                # tiles so each slot's readers finish before the ring wraps.
                s_ps = [ps.tile([P, D], F32, name="s_ps", tag="ps8", bufs=8)
                        for _ in range(2)]
                for lc in range(NL):
                    cols = slice(lc, lc + 1)
                    scols = slice(lc % 4, lc % 4 + 1)
                    sp = s_ps[lc // 4]
                    i = 0
                    for src, vc0 in ((tTs[b], V2C + b * NE),
                                     (xTs[b], V1C + b * NE)):
                        for k in range(NE):
                            nc.tensor.matmul(
                                sp[:, scols],
                                src[:, k, lc * P : (lc + 1) * P],
                                vecs[:, vc0 + k : vc0 + k + 1],
                                start=(i == 0), stop=(i == 2 * NE - 1))
                            i += 1
                    nc.vector.tensor_scalar_add(
                        rcolss[b][:, cols], sp[:, scols],
                        vecs[:, C0C + b : C0C + b + 1])
                    nc.vector.reciprocal(rcolss[b][:, cols], rcolss[b][:, cols])

            def ophase(b):
                # O = dT.T @ H, then (O * 1/ssum) + b3 fused on DVE with a
                # bf16 output (2 elem/cycle, half the store bytes)
                for lc in range(NL):
                    o_ps = ps.tile([P, D], F32, name="o_ps", tag="ps8", bufs=8)
                    for ec in range(NE):
                        nc.tensor.matmul(
                            o_ps, dTs[b][:, ec, lc * P : (lc + 1) * P],
                            H_sbs[b][:, ec, :], start=(ec == 0),
                            stop=(ec == NE - 1))
                    o_sb = opool.tile([P, D], BF16, name="o_sb")
                    nc.vector.scalar_tensor_tensor(
                        o_sb, o_ps, rcolss[b][:, lc : lc + 1], b3bc,
                        op0=ALU.mult, op1=ALU.add)
                    q = nc.sync if lc % 2 == 0 else nc.scalar
                    q.dma_start(
                        out=out_d[b, lc * P : (lc + 1) * P, :], in_=o_sb)

            # ---- interleaved schedule: batch 1's Gram/H fill batch 0's
            # load-paced dT gaps (PSUM ring verified slot-by-slot) ----
            tTr_copies(0)
            gram(0)
            hphase(0)
            xTr_copies(0)
            dt_t(0, 0)
            gram(1)
            dt_x(0, 0)
            hphase(1)
            dt_t(0, 1)
            dt_x(0, 1)
            ssum(0)
            tTr_copies(1)
            ophase(0)
            xTr_copies(1)
            dt_t(1, 0)
            dt_x(1, 0)
            dt_t(1, 1)
            dt_x(1, 1)
            ssum(1)
            ophase(1)

    nc.finalize()
    return nc


def _make_in_maps(in_seq, enc_seq, prev_target_seq, W_in2enc, b_in2enc,
                  W_lab2enc, b_lab2enc, W_enc2in, b_enc2in):
    f32 = np.float32
    f64 = np.float64
    W1 = np.asarray(W_in2enc, f32)
    W2 = np.asarray(W_lab2enc, f32)
    W3 = np.asarray(W_enc2in, f32)
    bd = (np.asarray(b_in2enc, f64) + np.asarray(b_lab2enc, f64)).astype(f32)
    w1t = np.ascontiguousarray(W1.T)   # [i, e]
    w2t = np.ascontiguousarray(W2.T)   # [j, e]
    w3tb = np.ascontiguousarray(W3.T.astype(BF16_NP))  # [e, o]
    b3bc = np.ascontiguousarray(
        np.broadcast_to(np.asarray(b_enc2in, f32), (P, D)))

    in_maps = []
    for c in range(N_CORES):
        bs = slice(c * BPC, (c + 1) * BPC)
        x = np.asarray(in_seq[:, bs, :], f32)
        t = np.asarray(prev_target_seq[:, bs, :], f32)
        e = np.asarray(enc_seq[:, bs, :], f32)
        # host glue: esum/v1/v2/c0 in fp64 (O(LBD + D^2) input/weight
        # reductions -- the ssum = X@v1 + T@v2 + c0 reformulation)
        esum = e.astype(f64).sum(axis=0)                    # [BPC, D]
        v1 = (esum @ W1.astype(f64)).astype(f32)            # [BPC, D]
        v2 = (esum @ W2.astype(f64)).astype(f32)
        c0 = (esum @ (np.asarray(b_in2enc, f64) + np.asarray(b_lab2enc, f64))
              ).astype(f32)                                  # [BPC]
        vecs = np.zeros((P, 22), f32)
        for b in range(BPC):
            vecs[:, V1C + b * NE : V1C + (b + 1) * NE] = v1[b].reshape(NE, P).T
            vecs[:, V2C + b * NE : V2C + (b + 1) * NE] = v2[b].reshape(NE, P).T
            vecs[:, C0C + b] = c0[b]
        vecs[:, BDC : BDC + NE] = bd.reshape(NE, P).T
        in_maps.append({
            "xT": np.ascontiguousarray(x.transpose(1, 2, 0)),
            "tT": np.ascontiguousarray(t.transpose(1, 2, 0)),
            "enb": np.ascontiguousarray(e.transpose(1, 0, 2).astype(BF16_NP)),
            "w1t": w1t, "w2t": w2t, "w3tb": w3tb,
            "vecs": vecs, "b3bc": b3bc,
        })
    return in_maps


_NC_CACHE = {}


def _get_nc():
    if "nc" not in _NC_CACHE:
        _NC_CACHE["nc"] = build_nc()
    return _NC_CACHE["nc"]


def kernel(**inputs):
    in_maps = _make_in_maps(**inputs)
    nc = _get_nc()
    res = run_bass_kernel_spmd(nc, in_maps, core_ids=list(range(N_CORES)))
    out = np.empty((L, B, D), np.float32)
    for c in range(N_CORES):
        per_core = np.asarray(res.results[c]["out"], np.float32)  # (BPC, L, D)
        for j in range(BPC):
            out[:, c * BPC + j, :] = per_core[j]
    return out


def kernel_sim(core_id=0, **inputs):
    """CoreSim validation path: simulate one core, return its (BPC, L, D) output."""
    from concourse.bass_interp import CoreSim

    in_maps = _make_in_maps(**inputs)
    nc = _get_nc()
    sim = CoreSim(nc)
    for name, val in in_maps[core_id].items():
        sim.tensor(name)[:] = val
    sim.simulate(check_with_hw=False)
    return np.array(sim.tensor("out")).astype(np.float32)
